# revision 7
# baseline (speedup 1.0000x reference)
"""CrossEncoderReranker TRN2 Bass kernel.

reference computation:
    x = concat([mention_embs[mention_idx], candidate_embs], 1)   # [T, 2H]
    h = relu(x @ W1 + b1)                                        # [T, H]
    s = (h @ W2 + b2)[:, 0]                                      # [T]
    out = scatter(s -> [N, MAXK] at (mention_idx, col_idx)) + 0.5 * faiss
    out = concat([out, nota_col], 1)                             # [N, MAXK+1]

Device strategy (8-way data parallel over contiguous mention ranges):
  * All matmuls run in bf16 (measured ~6% faster than f32r on the PE and
    half the DMA/SBUF footprint; end-to-end rel err ~3e-3).
  * x @ W1 = mention_part + candidate_part.  The candidate part is computed
    in h^T layout ([j, t], j on partitions) with W1 (bottom half) as the
    stationary operand.  The mention part A = mention_embs @ W1_top is
    computed ON HOST (numpy) and injected into the same PSUM accumulation
    via one-hot expansion matmuls (A_loc.T @ E, E[m, t] = [mention[t] ==
    base + m]).  E is built on host and shipped inside each tile's rhs slab
    (one contiguous DMA per tile: 6 candidate k-chunks + one E chunk per
    mention window).
  * relu(psum + b1[j]) runs on ACT with b1 as the per-partition bias; ht is
    written bf16 and reduced against W2 on the PE.
  * pair scores go to a DRAM scratch; the ragged->padded scatter is done as
    an indirect-DMA *gather* of overlapping 64-wide windows (row m starts at
    segment offset m), masked by a host-built col<len mask, then added to a
    host-scaled 0.5*faiss term on DVE.
"""

import sys

sys.path.insert(0, "/opt/trn_rl_repo")

from contextlib import ExitStack

import numpy as np

import concourse.bass as bass
import concourse.tile as tile
from concourse import mybir
from concourse.tile_rust import add_dep_helper

F32 = mybir.dt.float32
BF16 = mybir.dt.bfloat16
I32 = mybir.dt.int32
AF = mybir.ActivationFunctionType
ALU = mybir.AluOpType

N_CORES = 8
H = 768
P = 128
KC = H // P            # 6 k-chunks per 768
MAXK = 64
TT = 512               # candidate tile (and DMA slab) size
CHT = 8                # tiles per scores chunk


class SplitDrainTileContext(tile.TileContext):
    """The tail drain would carry one sync wait per logical proc; walrus caps
    sync waits per instruction. Absorb the global clock one proc at a time
    through SP NOPs (<=1 wait each), then emit the drain with a zero clock."""

    def _drain_and_barrier(self, tick_clock, wait_clock):
        from concourse.vector_clock import ScopedClock, VectorClock

        vals = list(tick_clock.global_clock)
        nprocs = len(vals)
        for q in range(nprocs):
            if not vals[q]:
                continue
            partial = [vals[p] if p == q else 0 for p in range(nprocs)]
            nop = self.nc.sync.nop()
            wait_clock.add_sem_waits(
                nop.ins, ScopedClock({None: VectorClock(partial)})
            )
        drain_inst = self.nc.sync.drain()
        wait_clock.add_sem_waits(
            drain_inst.ins, ScopedClock({None: VectorClock([0] * nprocs)})
        )
        self.nc.all_engine_barrier()
        popped = self.nc._tile_sem_poison_stack.pop()
        assert popped is self._sem_poison
        self.nc.clear_and_free_semaphores(list(self.sems.allocated().values()))
        self.nc.all_engine_barrier()


def split_waits(nc, cap=1):
    """This walrus build allows only ONE sync wait per instruction (two for
    some structs, but one is universally safe).  Move extra waits onto
    freshly inserted same-engine NOPs placed right before the instruction —
    the engine stalls at the NOP instead, semantics unchanged."""
    for fn in nc.m.functions:
        for bb in fn.blocks:
            new = []
            for inst in bb.instructions:
                si = inst.sync_info
                waits = list(si.on_wait) if si and si.on_wait else []
                if len(waits) > cap:
                    keep = waits[-cap:]
                    for k, wt in enumerate(waits[:-cap]):
                        nop = mybir.InstNoOp(
                            name=f"{inst.name}-wsp{k}",
                            engine=inst.engine,
                            ins=[], outs=[],
                            sync_info=mybir.SyncInfo(on_wait=[wt], on_update=[]),
                        )
                        nc.register_instruction(nop)
                        new.append(nop)
                    inst.sync_info = mybir.SyncInfo(
                        on_wait=keep, on_update=list(si.on_update or [])
                    )
                new.append(inst)
            bb.instructions = new


def build_program(T_pad, M_pad, windows, gdep):
    """One SPMD Bass program shared by all cores.

    windows[i]: sorted m-chunk indices whose mentions appear in candidate
    tile i on ANY core (union), so the program is core-independent.
    gdep[mc]: index of the scores-chunk DMA that must land before output
    chunk mc can be gathered (max over cores).
    """
    NT = T_pad // TT
    MC = M_pad // P
    assert len(windows) == NT
    assert len(gdep) == MC

    slab_w = [(KC + len(w)) * TT for w in windows]
    slab_off = np.concatenate([[0], np.cumsum(slab_w)]).astype(int)
    rhs_total = int(slab_off[-1])

    nc = bass.Bass()

    rhs = nc.dram_tensor("rhs", [P, rhs_total], BF16, kind="ExternalInput")
    w1 = nc.dram_tensor("w1", [P, KC * H], BF16, kind="ExternalInput")
    a_in = nc.dram_tensor("a_in", [P, MC * H], BF16, kind="ExternalInput")
    w2 = nc.dram_tensor("w2", [P, KC], BF16, kind="ExternalInput")
    b1 = nc.dram_tensor("b1", [P, KC], F32, kind="ExternalInput")
    b2 = nc.dram_tensor("b2", [1, 1], F32, kind="ExternalInput")
    offs = nc.dram_tensor("offs", [P, MC], I32, kind="ExternalInput")
    mask = nc.dram_tensor("mask", [P, MC * MAXK], F32, kind="ExternalInput")
    fh = nc.dram_tensor("fh", [P, MC * MAXK], F32, kind="ExternalInput")

    out = nc.dram_tensor("out", [M_pad, MAXK], F32, kind="ExternalOutput")
    sc_dram = nc.dram_tensor("sc_scratch", [T_pad + MAXK, 1], F32, kind="Internal")

    with ExitStack() as ctx:
        tc = ctx.enter_context(SplitDrainTileContext(nc))
        cst = ctx.enter_context(tc.tile_pool(name="cst", bufs=1))
        candp = ctx.enter_context(tc.tile_pool(name="candp", bufs=4))
        htp = ctx.enter_context(tc.tile_pool(name="htp", bufs=2))
        gp = ctx.enter_context(tc.tile_pool(name="gp", bufs=2))
        scp = ctx.enter_context(tc.tile_pool(name="scp", bufs=2))
        hps = ctx.enter_context(tc.tile_pool(name="hps", bufs=2, space="PSUM"))
        sps = ctx.enter_context(tc.tile_pool(name="sps", bufs=2, space="PSUM"))

        # ---- constants (DMA-ordered: GEMM-critical first) ----
        w1_sb = cst.tile([P, KC * H], BF16)
        nc.sync.dma_start(w1_sb[:], w1[:])
        a_sb = cst.tile([P, MC * H], BF16)
        nc.sync.dma_start(a_sb[:], a_in[:])
        w2_sb = cst.tile([P, KC], BF16)
        nc.sync.dma_start(w2_sb[:], w2[:])
        b1_sb = cst.tile([P, KC], F32)
        nc.sync.dma_start(b1_sb[:], b1[:])
        b2_sb = cst.tile([1, 1], F32)
        nc.sync.dma_start(b2_sb[:], b2[:])
        offs_sb = cst.tile([P, MC], I32)
        nc.sync.dma_start(offs_sb[:], offs[:])
        mask_sb = cst.tile([P, MC * MAXK], F32)
        nc.sync.dma_start(mask_sb[:], mask[:])
        fh_sb = cst.tile([P, MC * MAXK], F32)
        nc.sync.dma_start(fh_sb[:], fh[:])
        scratch_sb = cst.tile([1, 32], F32)

        def dummy_ldw(src_ap, dep_of=None):
            """bf16 ldweights reading 1 elem of src — absorbs one cross-engine
            wait into the PE clock (matmuls may carry only one wait)."""
            d = nc.tensor.ldweights(src_ap[0:1, 0:1].bitcast(BF16))
            if dep_of is not None:
                add_dep_helper(d.ins, dep_of.ins, reason="absorb wait")
            return d

        # absorb the constant-load DMA waits PE will otherwise inherit
        dummy_ldw(w1_sb)
        dummy_ldw(a_sb)
        dummy_ldw(w2_sb)
        # absorb b1/b2 DMA waits into ACT
        nc.scalar.activation(scratch_sb[0:1, 0:1], b1_sb[0:1, 0:1], AF.Identity)
        nc.scalar.activation(scratch_sb[0:1, 1:2], b2_sb[0:1, 0:1], AF.Identity)

        # PSUM-slot bookkeeping: a matmul that opens a new accumulation group
        # in a previously-used PSUM bank carries a self-PE wait (drain order)
        # plus a wait on the slot's last consumer — one too many for the
        # single-wait matmul encoding.  pe_absorb() soaks the consumer
        # wait into the PE clock first.
        hps_free = [None, None]
        hps_ctr = [0]

        def pe_absorb(dep_inst):
            if dep_inst is not None:
                dummy_ldw(w1_sb, dep_of=dep_inst)

        def acquire_hpsum():
            slot = hps_ctr[0] % 2
            hps_ctr[0] += 1
            pe_absorb(hps_free[slot])
            t = hps.tile([P, 3 * TT], F32, tag="hpsum")
            return t, slot

        # zero the gather-overread tail of the scores scratch
        z_t = cst.tile([1, MAXK], F32)
        nc.vector.memset(z_t[:], 0.0)
        sc_flat0 = sc_dram[:].rearrange("t a -> (a t)")[None, :]
        nc.sync.dma_start(sc_flat0[0:1, T_pad:T_pad + MAXK], z_t[0:1, :])

        # ---- ragged->padded gather + mask + faiss ----
        def emit_out_chunk(mc, sc_dma):
            g_t = gp.tile([P, MAXK], F32, tag="gath")
            gth = nc.gpsimd.indirect_dma_start(
                out=g_t[:], out_offset=None,
                in_=sc_dram[:],
                in_offset=bass.IndirectOffsetOnAxis(ap=offs_sb[:, mc:mc + 1], axis=0),
            )
            add_dep_helper(gth.ins, sc_dma.ins, reason="gather needs scores")
            gm_t = gp.tile([P, MAXK], F32, tag="gm")
            nc.vector.tensor_tensor(
                gm_t[:], g_t[:], mask_sb[:, mc * MAXK:(mc + 1) * MAXK], ALU.mult
            )
            o_t = gp.tile([P, MAXK], F32, tag="osb")
            nc.vector.tensor_tensor(
                o_t[:], gm_t[:], fh_sb[:, mc * MAXK:(mc + 1) * MAXK], ALU.add
            )
            nc.sync.dma_start(out[mc * P:(mc + 1) * P, :], o_t[:])

        # ---- main loop over candidate tiles ----
        # The W2 reduction matmul for group jc is *lagged*: it is emitted
        # ~4 candidate matmuls into the NEXT group, so the in-order PE queue
        # never stalls waiting for the ACT relu that produces ht[jc].  The
        # per-tile scores copy (and scores-chunk DMA + output gathers) are
        # likewise emitted right after that tile's lagged W2[5].
        CH = CHT * TT
        sps_free = [None, None]
        sc_t = None
        sc_tiles = {}                   # chunk index -> sbuf chunk tile
        sc_slot_dma = [None, None]      # chunk DMA that freed each scp slot
        pending_w2 = [None]             # (s_ps, jc, ht, sl)
        pending_tail = [None]           # tile index whose scores copy is due

        def flush_pending():
            if pending_w2[0] is None:
                return
            s_ps_p, jc_p, ht_p, sl_p = pending_w2[0]
            pending_w2[0] = None
            nc.tensor.matmul(
                s_ps_p[0:1, :],
                lhsT=w2_sb[:, jc_p:jc_p + 1],
                rhs=ht_p[:, sl_p],
                start=(jc_p == 0), stop=(jc_p == KC - 1),
            )
            if jc_p == KC - 1:
                ti = pending_tail[0]
                pending_tail[0] = None
                emit_tile_tail(ti, s_ps_p)

        def emit_tile_tail(ti, s_ps_p):
            sct = sc_tiles[ti // CHT]
            sps_free[ti % 2] = nc.scalar.activation(
                sct[0:1, (ti % CHT) * TT:(ti % CHT) * TT + TT], s_ps_p[0:1, :],
                AF.Identity, bias=b2_sb[0:1, 0:1],
            )
            if ti % CHT == CHT - 1 or ti == NT - 1:
                ci = ti // CHT
                c0 = ci * CH
                cn = min(CH, T_pad - c0)
                sc_flat = sc_dram[:].rearrange("t a -> (a t)")[None, :]
                d = nc.sync.dma_start(sc_flat[0:1, c0:c0 + cn], sct[0:1, 0:cn])
                sc_slot_dma[ci % 2] = d
                # emit output chunks whose score range is now complete
                for mc in range(MC):
                    if gdep[mc] == ci:
                        emit_out_chunk(mc, d)

        for i in range(NT):
            wl = len(windows[i])
            if i % CHT == 0:
                slot_c = (i // CHT) % 2
                prev_dma = sc_slot_dma[slot_c]
                sc_t = scp.tile([1, CH], F32, tag="scchunk")
                sc_tiles[i // CHT] = sc_t
                if prev_dma is not None:
                    ab = nc.scalar.activation(
                        scratch_sb[0:1, 2:3], b2_sb[0:1, 0:1], AF.Identity
                    )
                    add_dep_helper(ab.ins, prev_dma.ins,
                                   reason="absorb scores-chunk DMA wait into ACT")
            slab = candp.tile([P, (KC + wl) * TT], BF16, tag="slab")
            cdma = nc.sync.dma_start(
                slab[:], rhs[:, int(slab_off[i]):int(slab_off[i]) + (KC + wl) * TT]
            )
            d1 = dummy_ldw(slab, dep_of=cdma)

            for half in range(2):
                ps, slot = acquire_hpsum()
                ht = htp.tile([P, 3 * TT], BF16, tag="ht")
                last_relu = None
                for jj in range(3):
                    jc = half * 3 + jj
                    sl = slice(jj * TT, (jj + 1) * TT)
                    for kc in range(KC):
                        mm = nc.tensor.matmul(
                            ps[:, sl],
                            lhsT=w1_sb[:, kc * H + jc * P:kc * H + (jc + 1) * P],
                            rhs=slab[:, kc * TT:(kc + 1) * TT],
                            start=(kc == 0), stop=False,
                        )
                        if half == 0 and jj == 0 and kc == 0:
                            add_dep_helper(mm.ins, d1.ins, reason="order")
                        if kc == 3:
                            flush_pending()
                    for wi, w in enumerate(windows[i]):
                        nc.tensor.matmul(
                            ps[:, sl],
                            lhsT=a_sb[:, w * H + jc * P:w * H + (jc + 1) * P],
                            rhs=slab[:, (KC + wi) * TT:(KC + wi + 1) * TT],
                            start=False, stop=(wi == wl - 1),
                        )
                    last_relu = nc.scalar.activation(
                        ht[:, sl], ps[:, sl], AF.Relu, bias=b1_sb[:, jc:jc + 1]
                    )
                    # W2 reduction: s[0, t] += W2[jc].T @ relu_h[jc] (lagged)
                    if jc == 0:
                        pe_absorb(sps_free[i % 2])
                        s_ps = sps.tile([1, TT], F32, tag="spsum")
                    pending_w2[0] = (s_ps, jc, ht, sl)
                    if jc == KC - 1:
                        pending_tail[0] = i
                hps_free[slot] = last_relu
        flush_pending()

    split_waits(nc)
    return nc


def prepare(inputs):
    """Shard + lay out the full inputs; returns (build params, in_maps, meta)."""
    import ml_dtypes

    BF = ml_dtypes.bfloat16

    mention_embs = np.asarray(inputs["mention_embs"], dtype=np.float32)
    candidate_embs = np.asarray(inputs["candidate_embs"], dtype=np.float32)
    W1 = np.asarray(inputs["W1"], dtype=np.float32)
    b1 = np.asarray(inputs["b1"], dtype=np.float32)
    W2 = np.asarray(inputs["W2"], dtype=np.float32)
    b2 = np.asarray(inputs["b2"], dtype=np.float32)
    faiss_prior = np.asarray(inputs["faiss_prior"], dtype=np.float32)
    mention_idx = np.asarray(inputs["mention_idx"], dtype=np.int64)
    col_idx = np.asarray(inputs["col_idx"], dtype=np.int64)

    N = mention_embs.shape[0]
    T = mention_idx.shape[0]
    assert np.all(np.diff(mention_idx) >= 0), "mention_idx must be sorted"
    lengths = np.bincount(mention_idx, minlength=N)
    offsets = np.concatenate([[0], np.cumsum(lengths)[:-1]])
    # col_idx must be arange within each contiguous segment
    assert np.array_equal(col_idx, np.arange(T) - np.repeat(offsets, lengths))

    # mention part of x @ W1, computed on host
    A = mention_embs @ W1[:H]                       # [N, H] f32

    # split mentions into 8 contiguous groups with ~equal candidate counts
    cum = np.cumsum(lengths)
    bnd = [0]
    for c in range(1, N_CORES):
        b = int(np.searchsorted(cum, c * T / N_CORES))
        bnd.append(max(bnd[-1] + 1, min(b + 1, N - (N_CORES - c))))
    bnd.append(N)

    T_cs = [int(cum[bnd[c + 1] - 1] - (cum[bnd[c] - 1] if bnd[c] else 0))
            for c in range(N_CORES)]
    M_cs = [bnd[c + 1] - bnd[c] for c in range(N_CORES)]
    T_pad = -(-max(T_cs) // TT) * TT
    M_pad = -(-max(M_cs) // P) * P
    NT, MC = T_pad // TT, M_pad // P

    # per-tile m-chunk windows, unioned across cores
    windows = [set() for _ in range(NT)]
    core_data = []
    for c in range(N_CORES):
        m0, m1 = bnd[c], bnd[c + 1]
        t0 = int(offsets[m0])
        T_c, M_c = T_cs[c], M_cs[c]
        ml = (mention_idx[t0:t0 + T_c] - m0).astype(np.int64)
        for i in range(NT):
            seg = ml[i * TT:(i + 1) * TT]
            if seg.size:
                for w in np.unique(seg // P):
                    windows[i].add(int(w))
        core_data.append((m0, m1, t0, T_c, M_c, ml))
    windows = [sorted(w) if w else [0] for w in windows]

    # gather dependency: which scores-chunk DMA must land before output
    # chunk mc can be gathered — max over cores
    CH = CHT * TT
    n_chunks = (NT + CHT - 1) // CHT
    gdep = [0] * MC
    for c in range(N_CORES):
        m0, m1, t0, T_c, M_c, ml = core_data[c]
        offs_c = (offsets[m0:m1] - t0).astype(np.int64)
        for mc in range(MC):
            rows = offs_c[mc * P:(mc + 1) * P]
            if rows.size == 0:
                continue
            end = min(int(rows.max()) + MAXK, T_pad)
            k = min((end - 1) // CH, n_chunks - 1)
            gdep[mc] = max(gdep[mc], k)

    slab_w = [(KC + len(w)) * TT for w in windows]
    slab_off = np.concatenate([[0], np.cumsum(slab_w)]).astype(int)
    rhs_total = int(slab_off[-1])

    # shared (replicated) tensors
    w1_l = np.ascontiguousarray(
        W1[H:].reshape(KC, P, H).transpose(1, 0, 2).reshape(P, KC * H)).astype(BF)
    w2_l = np.ascontiguousarray(W2[:, 0].reshape(KC, P).T).astype(BF)
    b1_l = np.ascontiguousarray(b1.reshape(KC, P).T)
    b2_l = b2.reshape(1, 1)
    iota64 = np.arange(MAXK, dtype=np.int64)

    in_maps = []
    for c in range(N_CORES):
        m0, m1, t0, T_c, M_c, ml = core_data[c]
        rhs_l = np.zeros((P, rhs_total), dtype=BF)
        cT = candidate_embs[t0:t0 + T_c].T.astype(BF)    # [H, T_c]
        for i in range(NT):
            base = int(slab_off[i])
            tt0 = i * TT
            tn = max(0, min(TT, T_c - tt0))
            if tn > 0:
                blk = cT[:, tt0:tt0 + tn].reshape(KC, P, tn)
                for kc in range(KC):
                    rhs_l[:, base + kc * TT:base + kc * TT + tn] = blk[kc]
                # one-hot E chunks
                seg = ml[tt0:tt0 + tn]
                tloc = np.arange(tn)
                for wi, w in enumerate(windows[i]):
                    rows = seg - w * P
                    sel = (rows >= 0) & (rows < P)
                    eb = base + (KC + wi) * TT
                    rhs_l[rows[sel], eb + tloc[sel]] = BF(1.0)

        A_l = np.zeros((M_pad, H), dtype=np.float32)
        A_l[:M_c] = A[m0:m1]
        a_l = np.ascontiguousarray(
            A_l.reshape(MC, P, H).transpose(1, 0, 2).reshape(P, MC * H)).astype(BF)

        offs_l = np.zeros(M_pad, dtype=np.int32)
        offs_l[:M_c] = (offsets[m0:m1] - t0).astype(np.int32)
        lens_l = np.zeros(M_pad, dtype=np.int64)
        lens_l[:M_c] = lengths[m0:m1]
        mask_l = (iota64[None, :] < lens_l[:, None]).astype(np.float32)
        fh_l = np.zeros((M_pad, MAXK), dtype=np.float32)
        fh_l[:M_c] = 0.5 * faiss_prior[m0:m1]
        in_maps.append({
            "rhs": rhs_l,
            "w1": w1_l, "a_in": a_l, "w2": w2_l, "b1": b1_l, "b2": b2_l,
            "offs": np.ascontiguousarray(offs_l.reshape(MC, P).T),
            "mask": np.ascontiguousarray(
                mask_l.reshape(MC, P, MAXK).transpose(1, 0, 2).reshape(P, MC * MAXK)),
            "fh": np.ascontiguousarray(
                fh_l.reshape(MC, P, MAXK).transpose(1, 0, 2).reshape(P, MC * MAXK)),
        })
    return (T_pad, M_pad, windows, gdep), in_maps, (bnd, N)


def assemble(results, meta, nota_bias):
    bnd, N = meta
    out = np.empty((N, MAXK + 1), dtype=np.float32)
    for c in range(N_CORES):
        m0, m1 = bnd[c], bnd[c + 1]
        out[m0:m1, :MAXK] = results[c]["out"][:m1 - m0]
    out[:, MAXK] = np.float32(nota_bias)
    return out


_CACHE = {}


def kernel(**inputs) -> np.ndarray:
    from concourse.bass_utils import run_bass_kernel_spmd

    (T_pad, M_pad, windows, gdep), in_maps, meta = prepare(inputs)
    key = (T_pad, M_pad, tuple(tuple(w) for w in windows), tuple(gdep))
    if key not in _CACHE:
        _CACHE[key] = build_program(T_pad, M_pad, windows, gdep)
    nc = _CACHE[key]
    res = run_bass_kernel_spmd(nc, in_maps, list(range(N_CORES)))
    return assemble(res.results, meta, np.asarray(inputs["nota_bias"]))


# revision 9
# speedup vs baseline: 1.1424x; 1.1424x over previous
"""CrossEncoderReranker TRN2 Bass kernel.

reference computation:
    x = concat([mention_embs[mention_idx], candidate_embs], 1)   # [T, 2H]
    h = relu(x @ W1 + b1)                                        # [T, H]
    s = (h @ W2 + b2)[:, 0]                                      # [T]
    out = scatter(s -> [N, MAXK] at (mention_idx, col_idx)) + 0.5 * faiss
    out = concat([out, nota_col], 1)                             # [N, MAXK+1]

Device strategy (8-way data parallel over contiguous mention ranges):
  * All matmuls run in bf16 (measured ~6% faster than f32r on the PE and
    half the DMA/SBUF footprint; end-to-end rel err ~3e-3).
  * x @ W1 = mention_part + candidate_part.  The candidate part is computed
    in h^T layout ([j, t], j on partitions) with W1 (bottom half) as the
    stationary operand.  The mention part A = mention_embs @ W1_top is
    computed ON HOST (numpy) and injected into the same PSUM accumulation
    via one-hot expansion matmuls (A_loc.T @ E, E[m, t] = [mention[t] ==
    base + m]).  E is built on host and shipped inside each tile's rhs slab
    (one contiguous DMA per tile: 6 candidate k-chunks + one E chunk per
    mention window).
  * relu(psum + b1[j]) runs on ACT with b1 as the per-partition bias; ht is
    written bf16 and reduced against W2 on the PE.
  * pair scores go to a DRAM scratch; the ragged->padded scatter is done as
    an indirect-DMA *gather* of overlapping 64-wide windows (row m starts at
    segment offset m), masked by a host-built col<len mask, then added to a
    host-scaled 0.5*faiss term on DVE.
"""

import sys

sys.path.insert(0, "/opt/trn_rl_repo")

from contextlib import ExitStack

import numpy as np

import concourse.bass as bass
import concourse.tile as tile
from concourse import mybir
from concourse.tile_rust import add_dep_helper

F32 = mybir.dt.float32
BF16 = mybir.dt.bfloat16
I32 = mybir.dt.int32
AF = mybir.ActivationFunctionType
ALU = mybir.AluOpType

N_CORES = 8
H = 768
P = 128
KC = H // P            # 6 k-chunks per 768
MAXK = 64
TT = 512               # candidate tile (and DMA slab) size
CHT = 8                # tiles per scores chunk


class SplitDrainTileContext(tile.TileContext):
    """The tail drain would carry one sync wait per logical proc; walrus caps
    sync waits per instruction. Absorb the global clock one proc at a time
    through SP NOPs (<=1 wait each), then emit the drain with a zero clock."""

    def _drain_and_barrier(self, tick_clock, wait_clock):
        from concourse.vector_clock import ScopedClock, VectorClock

        vals = list(tick_clock.global_clock)
        nprocs = len(vals)
        for q in range(nprocs):
            if not vals[q]:
                continue
            partial = [vals[p] if p == q else 0 for p in range(nprocs)]
            nop = self.nc.sync.nop()
            wait_clock.add_sem_waits(
                nop.ins, ScopedClock({None: VectorClock(partial)})
            )
        drain_inst = self.nc.sync.drain()
        wait_clock.add_sem_waits(
            drain_inst.ins, ScopedClock({None: VectorClock([0] * nprocs)})
        )
        self.nc.all_engine_barrier()
        popped = self.nc._tile_sem_poison_stack.pop()
        assert popped is self._sem_poison
        self.nc.clear_and_free_semaphores(list(self.sems.allocated().values()))
        self.nc.all_engine_barrier()


def split_waits(nc, cap=1):
    """This walrus build allows only ONE sync wait per instruction (two for
    some structs, but one is universally safe).  Move extra waits onto
    freshly inserted same-engine NOPs placed right before the instruction —
    the engine stalls at the NOP instead, semantics unchanged."""
    for fn in nc.m.functions:
        for bb in fn.blocks:
            new = []
            for inst in bb.instructions:
                si = inst.sync_info
                waits = list(si.on_wait) if si and si.on_wait else []
                if len(waits) > cap:
                    keep = waits[-cap:]
                    for k, wt in enumerate(waits[:-cap]):
                        nop = mybir.InstNoOp(
                            name=f"{inst.name}-wsp{k}",
                            engine=inst.engine,
                            ins=[], outs=[],
                            sync_info=mybir.SyncInfo(on_wait=[wt], on_update=[]),
                        )
                        nc.register_instruction(nop)
                        new.append(nop)
                    inst.sync_info = mybir.SyncInfo(
                        on_wait=keep, on_update=list(si.on_update or [])
                    )
                new.append(inst)
            bb.instructions = new


def build_program(T_pad, M_pad, windows, gdep):
    """One SPMD Bass program shared by all cores.

    windows[i]: sorted m-chunk indices whose mentions appear in candidate
    tile i on ANY core (union), so the program is core-independent.
    gdep[mc]: index of the scores-chunk DMA that must land before output
    chunk mc can be gathered (max over cores).
    """
    NT = T_pad // TT
    MC = M_pad // P
    assert len(windows) == NT
    assert len(gdep) == MC

    slab_w = [(KC + len(w)) * TT for w in windows]
    slab_off = np.concatenate([[0], np.cumsum(slab_w)]).astype(int)
    rhs_total = int(slab_off[-1])

    nc = bass.Bass()

    rhs = nc.dram_tensor("rhs", [P, rhs_total], BF16, kind="ExternalInput")
    w1 = nc.dram_tensor("w1", [P, KC * H], BF16, kind="ExternalInput")
    a_in = nc.dram_tensor("a_in", [P, MC * H], BF16, kind="ExternalInput")
    w2 = nc.dram_tensor("w2", [P, KC], BF16, kind="ExternalInput")
    b1 = nc.dram_tensor("b1", [P, KC], F32, kind="ExternalInput")
    b2 = nc.dram_tensor("b2", [1, 1], F32, kind="ExternalInput")
    offs = nc.dram_tensor("offs", [P, MC], I32, kind="ExternalInput")
    mask = nc.dram_tensor("mask", [P, MC * MAXK], F32, kind="ExternalInput")
    fh = nc.dram_tensor("fh", [P, MC * MAXK], F32, kind="ExternalInput")

    out = nc.dram_tensor("out", [M_pad, MAXK], F32, kind="ExternalOutput")
    sc_dram = nc.dram_tensor("sc_scratch", [T_pad + MAXK, 1], F32, kind="Internal")

    with ExitStack() as ctx:
        tc = ctx.enter_context(SplitDrainTileContext(nc))
        cst = ctx.enter_context(tc.tile_pool(name="cst", bufs=1))
        candp = ctx.enter_context(tc.tile_pool(name="candp", bufs=4))
        htp = ctx.enter_context(tc.tile_pool(name="htp", bufs=4))
        gp = ctx.enter_context(tc.tile_pool(name="gp", bufs=2))
        scp = ctx.enter_context(tc.tile_pool(name="scp", bufs=2))
        hps = ctx.enter_context(tc.tile_pool(name="hps", bufs=2, space="PSUM"))
        sps = ctx.enter_context(tc.tile_pool(name="sps", bufs=2, space="PSUM"))

        # ---- constants (DMA-ordered: GEMM-critical first) ----
        w1_sb = cst.tile([P, KC * H], BF16)
        nc.sync.dma_start(w1_sb[:], w1[:])
        w2_sb = cst.tile([P, KC], BF16)
        nc.sync.dma_start(w2_sb[:], w2[:])
        b1_sb = cst.tile([P, KC], F32)
        nc.sync.dma_start(b1_sb[:], b1[:])
        b2_sb = cst.tile([1, 1], F32)
        nc.sync.dma_start(b2_sb[:], b2[:])
        offs_sb = cst.tile([P, MC], I32)
        nc.sync.dma_start(offs_sb[:], offs[:])
        a_sb = cst.tile([P, MC * H], BF16)
        nc.sync.dma_start(a_sb[:], a_in[:])

        # pre-issue the first few candidate slab DMAs ahead of the cold
        # constants below so the PE can start as early as possible
        pre_slabs = {}
        for i in range(min(3, NT)):
            wl = len(windows[i])
            slab_t = candp.tile([P, (KC + wl) * TT], BF16, tag="slab")
            cd = nc.sync.dma_start(
                slab_t[:],
                rhs[:, int(slab_off[i]):int(slab_off[i]) + (KC + wl) * TT],
            )
            pre_slabs[i] = (slab_t, cd)

        mask_sb = cst.tile([P, MC * MAXK], F32)
        nc.sync.dma_start(mask_sb[:], mask[:])
        fh_sb = cst.tile([P, MC * MAXK], F32)
        nc.sync.dma_start(fh_sb[:], fh[:])
        scratch_sb = cst.tile([1, 32], F32)

        def dummy_ldw(src_ap, dep_of=None):
            """bf16 ldweights reading 1 elem of src — absorbs one cross-engine
            wait into the PE clock (matmuls may carry only one wait)."""
            d = nc.tensor.ldweights(src_ap[0:1, 0:1].bitcast(BF16))
            if dep_of is not None:
                add_dep_helper(d.ins, dep_of.ins, reason="absorb wait")
            return d

        # absorb the constant-load DMA waits PE will otherwise inherit
        dummy_ldw(w1_sb)
        dummy_ldw(a_sb)
        dummy_ldw(w2_sb)
        # absorb b1/b2 DMA waits into ACT
        nc.scalar.activation(scratch_sb[0:1, 0:1], b1_sb[0:1, 0:1], AF.Identity)
        nc.scalar.activation(scratch_sb[0:1, 1:2], b2_sb[0:1, 0:1], AF.Identity)

        # PSUM-slot bookkeeping: a matmul that opens a new accumulation group
        # in a previously-used PSUM bank carries a self-PE wait (drain order)
        # plus a wait on the slot's last consumer — one too many for the
        # single-wait matmul encoding.  pe_absorb() soaks the consumer
        # wait into the PE clock first.
        hps_free = [None, None]
        hps_ctr = [0]

        def pe_absorb(dep_inst):
            if dep_inst is not None:
                dummy_ldw(w1_sb, dep_of=dep_inst)

        def acquire_hpsum():
            slot = hps_ctr[0] % 2
            hps_ctr[0] += 1
            pe_absorb(hps_free[slot])
            t = hps.tile([P, 3 * TT], F32, tag="hpsum")
            return t, slot

        # zero the gather-overread tail of the scores scratch
        z_t = cst.tile([1, MAXK], F32)
        nc.vector.memset(z_t[:], 0.0)
        sc_flat0 = sc_dram[:].rearrange("t a -> (a t)")[None, :]
        nc.sync.dma_start(sc_flat0[0:1, T_pad:T_pad + MAXK], z_t[0:1, :])

        # ---- ragged->padded gather + mask + faiss ----
        def emit_out_chunk(mc, sc_dma):
            g_t = gp.tile([P, MAXK], F32, tag="gath")
            gth = nc.gpsimd.indirect_dma_start(
                out=g_t[:], out_offset=None,
                in_=sc_dram[:],
                in_offset=bass.IndirectOffsetOnAxis(ap=offs_sb[:, mc:mc + 1], axis=0),
            )
            add_dep_helper(gth.ins, sc_dma.ins, reason="gather needs scores")
            gm_t = gp.tile([P, MAXK], F32, tag="gm")
            nc.vector.tensor_tensor(
                gm_t[:], g_t[:], mask_sb[:, mc * MAXK:(mc + 1) * MAXK], ALU.mult
            )
            o_t = gp.tile([P, MAXK], F32, tag="osb")
            nc.vector.tensor_tensor(
                o_t[:], gm_t[:], fh_sb[:, mc * MAXK:(mc + 1) * MAXK], ALU.add
            )
            nc.sync.dma_start(out[mc * P:(mc + 1) * P, :], o_t[:])

        # ---- main loop over candidate tiles ----
        # The W2 reduction matmul for group jc is *lagged*: it is emitted
        # ~4 candidate matmuls into the NEXT group, so the in-order PE queue
        # never stalls waiting for the ACT relu that produces ht[jc].  The
        # per-tile scores copy (and scores-chunk DMA + output gathers) are
        # likewise emitted right after that tile's lagged W2[5].
        CH = CHT * TT
        sps_free = [None, None]
        sc_t = None
        sc_tiles = {}                   # chunk index -> sbuf chunk tile
        sc_slot_dma = [None, None]      # chunk DMA that freed each scp slot
        pending_w2 = [None]             # (tile, [ht_half0, ht_half1])

        def flush_pending():
            """Emit the 6 W2 reduction matmuls for the previous tile as one
            batch, far enough behind the relus that ACT latency is hidden."""
            if pending_w2[0] is None:
                return
            ti, hts = pending_w2[0]
            pending_w2[0] = None
            pe_absorb(sps_free[ti % 2])
            s_ps = sps.tile([1, TT], F32, tag="spsum")
            for jc in range(KC):
                nc.tensor.matmul(
                    s_ps[0:1, :],
                    lhsT=w2_sb[:, jc:jc + 1],
                    rhs=hts[jc // 3][:, (jc % 3) * TT:(jc % 3 + 1) * TT],
                    start=(jc == 0), stop=(jc == KC - 1),
                )
            emit_tile_tail(ti, s_ps)

        def emit_tile_tail(ti, s_ps_p):
            sct = sc_tiles[ti // CHT]
            sps_free[ti % 2] = nc.scalar.activation(
                sct[0:1, (ti % CHT) * TT:(ti % CHT) * TT + TT], s_ps_p[0:1, :],
                AF.Identity, bias=b2_sb[0:1, 0:1],
            )
            if ti % CHT == CHT - 1 or ti == NT - 1:
                ci = ti // CHT
                c0 = ci * CH
                cn = min(CH, T_pad - c0)
                sc_flat = sc_dram[:].rearrange("t a -> (a t)")[None, :]
                d = nc.sync.dma_start(sc_flat[0:1, c0:c0 + cn], sct[0:1, 0:cn])
                sc_slot_dma[ci % 2] = d
                # emit output chunks whose score range is now complete
                for mc in range(MC):
                    if gdep[mc] == ci:
                        emit_out_chunk(mc, d)

        for i in range(NT):
            wl = len(windows[i])
            if i % CHT == 0:
                slot_c = (i // CHT) % 2
                prev_dma = sc_slot_dma[slot_c]
                sc_t = scp.tile([1, CH], F32, tag="scchunk")
                sc_tiles[i // CHT] = sc_t
                if prev_dma is not None:
                    ab = nc.scalar.activation(
                        scratch_sb[0:1, 2:3], b2_sb[0:1, 0:1], AF.Identity
                    )
                    add_dep_helper(ab.ins, prev_dma.ins,
                                   reason="absorb scores-chunk DMA wait into ACT")
            if i in pre_slabs:
                slab, cdma = pre_slabs[i]
            else:
                slab = candp.tile([P, (KC + wl) * TT], BF16, tag="slab")
                cdma = nc.sync.dma_start(
                    slab[:],
                    rhs[:, int(slab_off[i]):int(slab_off[i]) + (KC + wl) * TT],
                )
            d1 = dummy_ldw(slab, dep_of=cdma)

            ht_halves = []
            for half in range(2):
                ps, slot = acquire_hpsum()
                ht = htp.tile([P, 3 * TT], BF16, tag="ht")
                last_relu = None
                for jj in range(3):
                    jc = half * 3 + jj
                    sl = slice(jj * TT, (jj + 1) * TT)
                    for kc in range(KC):
                        mm = nc.tensor.matmul(
                            ps[:, sl],
                            lhsT=w1_sb[:, kc * H + jc * P:kc * H + (jc + 1) * P],
                            rhs=slab[:, kc * TT:(kc + 1) * TT],
                            start=(kc == 0), stop=False,
                        )
                        if half == 0 and jj == 0 and kc == 0:
                            add_dep_helper(mm.ins, d1.ins, reason="order")
                    for wi, w in enumerate(windows[i]):
                        nc.tensor.matmul(
                            ps[:, sl],
                            lhsT=a_sb[:, w * H + jc * P:w * H + (jc + 1) * P],
                            rhs=slab[:, (KC + wi) * TT:(KC + wi + 1) * TT],
                            start=False, stop=(wi == wl - 1),
                        )
                    if half == 0 and jj == 0:
                        flush_pending()
                    last_relu = nc.scalar.activation(
                        ht[:, sl], ps[:, sl], AF.Relu, bias=b1_sb[:, jc:jc + 1]
                    )
                hps_free[slot] = last_relu
                ht_halves.append(ht)
            pending_w2[0] = (i, ht_halves)
        flush_pending()

    split_waits(nc)
    return nc


def prepare(inputs):
    """Shard + lay out the full inputs; returns (build params, in_maps, meta)."""
    import ml_dtypes

    BF = ml_dtypes.bfloat16

    mention_embs = np.asarray(inputs["mention_embs"], dtype=np.float32)
    candidate_embs = np.asarray(inputs["candidate_embs"], dtype=np.float32)
    W1 = np.asarray(inputs["W1"], dtype=np.float32)
    b1 = np.asarray(inputs["b1"], dtype=np.float32)
    W2 = np.asarray(inputs["W2"], dtype=np.float32)
    b2 = np.asarray(inputs["b2"], dtype=np.float32)
    faiss_prior = np.asarray(inputs["faiss_prior"], dtype=np.float32)
    mention_idx = np.asarray(inputs["mention_idx"], dtype=np.int64)
    col_idx = np.asarray(inputs["col_idx"], dtype=np.int64)

    N = mention_embs.shape[0]
    T = mention_idx.shape[0]
    assert np.all(np.diff(mention_idx) >= 0), "mention_idx must be sorted"
    lengths = np.bincount(mention_idx, minlength=N)
    offsets = np.concatenate([[0], np.cumsum(lengths)[:-1]])
    # col_idx must be arange within each contiguous segment
    assert np.array_equal(col_idx, np.arange(T) - np.repeat(offsets, lengths))

    # mention part of x @ W1, computed on host
    A = mention_embs @ W1[:H]                       # [N, H] f32

    # split mentions into 8 contiguous groups with ~equal candidate counts
    cum = np.cumsum(lengths)
    bnd = [0]
    for c in range(1, N_CORES):
        b = int(np.searchsorted(cum, c * T / N_CORES))
        bnd.append(max(bnd[-1] + 1, min(b + 1, N - (N_CORES - c))))
    bnd.append(N)

    T_cs = [int(cum[bnd[c + 1] - 1] - (cum[bnd[c] - 1] if bnd[c] else 0))
            for c in range(N_CORES)]
    M_cs = [bnd[c + 1] - bnd[c] for c in range(N_CORES)]
    T_pad = -(-max(T_cs) // TT) * TT
    M_pad = -(-max(M_cs) // P) * P
    NT, MC = T_pad // TT, M_pad // P

    # per-tile m-chunk windows, unioned across cores
    windows = [set() for _ in range(NT)]
    core_data = []
    for c in range(N_CORES):
        m0, m1 = bnd[c], bnd[c + 1]
        t0 = int(offsets[m0])
        T_c, M_c = T_cs[c], M_cs[c]
        ml = (mention_idx[t0:t0 + T_c] - m0).astype(np.int64)
        for i in range(NT):
            seg = ml[i * TT:(i + 1) * TT]
            if seg.size:
                for w in np.unique(seg // P):
                    windows[i].add(int(w))
        core_data.append((m0, m1, t0, T_c, M_c, ml))
    windows = [sorted(w) if w else [0] for w in windows]

    # gather dependency: which scores-chunk DMA must land before output
    # chunk mc can be gathered — max over cores
    CH = CHT * TT
    n_chunks = (NT + CHT - 1) // CHT
    gdep = [0] * MC
    for c in range(N_CORES):
        m0, m1, t0, T_c, M_c, ml = core_data[c]
        offs_c = (offsets[m0:m1] - t0).astype(np.int64)
        for mc in range(MC):
            rows = offs_c[mc * P:(mc + 1) * P]
            if rows.size == 0:
                continue
            end = min(int(rows.max()) + MAXK, T_pad)
            k = min((end - 1) // CH, n_chunks - 1)
            gdep[mc] = max(gdep[mc], k)

    slab_w = [(KC + len(w)) * TT for w in windows]
    slab_off = np.concatenate([[0], np.cumsum(slab_w)]).astype(int)
    rhs_total = int(slab_off[-1])

    # shared (replicated) tensors
    w1_l = np.ascontiguousarray(
        W1[H:].reshape(KC, P, H).transpose(1, 0, 2).reshape(P, KC * H)).astype(BF)
    w2_l = np.ascontiguousarray(W2[:, 0].reshape(KC, P).T).astype(BF)
    b1_l = np.ascontiguousarray(b1.reshape(KC, P).T)
    b2_l = b2.reshape(1, 1)
    iota64 = np.arange(MAXK, dtype=np.int64)

    in_maps = []
    for c in range(N_CORES):
        m0, m1, t0, T_c, M_c, ml = core_data[c]
        rhs_l = np.zeros((P, rhs_total), dtype=BF)
        cT = candidate_embs[t0:t0 + T_c].T.astype(BF)    # [H, T_c]
        for i in range(NT):
            base = int(slab_off[i])
            tt0 = i * TT
            tn = max(0, min(TT, T_c - tt0))
            if tn > 0:
                blk = cT[:, tt0:tt0 + tn].reshape(KC, P, tn)
                for kc in range(KC):
                    rhs_l[:, base + kc * TT:base + kc * TT + tn] = blk[kc]
                # one-hot E chunks
                seg = ml[tt0:tt0 + tn]
                tloc = np.arange(tn)
                for wi, w in enumerate(windows[i]):
                    rows = seg - w * P
                    sel = (rows >= 0) & (rows < P)
                    eb = base + (KC + wi) * TT
                    rhs_l[rows[sel], eb + tloc[sel]] = BF(1.0)

        A_l = np.zeros((M_pad, H), dtype=np.float32)
        A_l[:M_c] = A[m0:m1]
        a_l = np.ascontiguousarray(
            A_l.reshape(MC, P, H).transpose(1, 0, 2).reshape(P, MC * H)).astype(BF)

        offs_l = np.zeros(M_pad, dtype=np.int32)
        offs_l[:M_c] = (offsets[m0:m1] - t0).astype(np.int32)
        lens_l = np.zeros(M_pad, dtype=np.int64)
        lens_l[:M_c] = lengths[m0:m1]
        mask_l = (iota64[None, :] < lens_l[:, None]).astype(np.float32)
        fh_l = np.zeros((M_pad, MAXK), dtype=np.float32)
        fh_l[:M_c] = 0.5 * faiss_prior[m0:m1]
        in_maps.append({
            "rhs": rhs_l,
            "w1": w1_l, "a_in": a_l, "w2": w2_l, "b1": b1_l, "b2": b2_l,
            "offs": np.ascontiguousarray(offs_l.reshape(MC, P).T),
            "mask": np.ascontiguousarray(
                mask_l.reshape(MC, P, MAXK).transpose(1, 0, 2).reshape(P, MC * MAXK)),
            "fh": np.ascontiguousarray(
                fh_l.reshape(MC, P, MAXK).transpose(1, 0, 2).reshape(P, MC * MAXK)),
        })
    return (T_pad, M_pad, windows, gdep), in_maps, (bnd, N)


def assemble(results, meta, nota_bias):
    bnd, N = meta
    out = np.empty((N, MAXK + 1), dtype=np.float32)
    for c in range(N_CORES):
        m0, m1 = bnd[c], bnd[c + 1]
        out[m0:m1, :MAXK] = results[c]["out"][:m1 - m0]
    out[:, MAXK] = np.float32(nota_bias)
    return out


_CACHE = {}


def kernel(**inputs) -> np.ndarray:
    from concourse.bass_utils import run_bass_kernel_spmd

    (T_pad, M_pad, windows, gdep), in_maps, meta = prepare(inputs)
    key = (T_pad, M_pad, tuple(tuple(w) for w in windows), tuple(gdep))
    if key not in _CACHE:
        _CACHE[key] = build_program(T_pad, M_pad, windows, gdep)
    nc = _CACHE[key]
    res = run_bass_kernel_spmd(nc, in_maps, list(range(N_CORES)))
    return assemble(res.results, meta, np.asarray(inputs["nota_bias"]))
